# revision 29
# baseline (speedup 1.0000x reference)
"""Trainium2 Bass kernel for ConditionalSimNet2 (moe_routing).

Computation (B=128, FEAT_IN=2048, D=1024, N=P=66 conditions):
    x          = image @ W_emb + b_emb                    [B, D]
    masked_rep = einsum('bd,nde->bne', x, W_rep) + b_rep  [B, N, D]
    embed      = mask_table * masked_rep                  [B, N, D]
    att        = softmax(relu(cat_enc@W1+b1)@W2 + b2)     [P, N]
    cond_feat  = einsum('pn,bnd->bpd', att, embed)        [B, P, D]
    out        = concat([cond_feat, broadcast(x)], 1)     [B, P+N, D]

Device work is only the big GEMMs; everything input-only is host math:
  - mask_table is folded into W_rep columns / b_rep on the host.
  - att (66x66, input-only) is computed on the host; the device receives
    attT72 = 8*att permuted into exchange-row order.
  - b_rep's contribution att@ (mask*b_rep) is a batch-independent [P, D]
    matrix added on the host (it is exactly zero for this model).
  - b_emb rides as a 17th k-tile of the x GEMM (host-padded W_emb/imgT).

Sharding: expert-parallel over 66->72 conditions, 9 per core.  Every
core computes x redundantly (bf16), runs its 9 grouped GEMMs in fp8
DoubleRow (W pre-scaled x16 into e4m3 on host), exchanges embed slices
in fp8 via 3 pipelined AllToAlls (a tiny warm-up AllToAll at t=0
absorbs core launch skew), then reduces its 16-row batch shard with a
single fp8 matmul per 512-col slice (PSUM = 128*cond_feat, descaled in
the PSUM->SBUF copy).  Host concatenates the batch shards and
broadcasts x into the feature_x half.
"""

import os
import sys

import numpy as np

try:
    import concourse.bass as bass
except ImportError:  # pragma: no cover - fallback when PYTHONPATH is not set
    sys.path.insert(0, "/opt/trn_rl_repo")
    import concourse.bass as bass

import concourse.mybir as mybir
import concourse.tile as tile
from concourse.bass_utils import run_bass_kernel_spmd

F32 = mybir.dt.float32
BF16 = mybir.dt.bfloat16
FP8 = mybir.dt.float8e4

B = 128          # batch
FI = 2048        # backbone feature dim
D = 1024         # embed dim
N = 66           # conditions (== pair categories P)
P = 66
NCORES = 8
NL = 9           # conditions per core (66 -> 72 padded)
NPAD = NCORES * NL
BL = B // NCORES  # batch rows per core
KF = FI // 128 + 1  # 16 k-tiles over FEAT_IN + 1 bias tile
KD = D // 128       # 8 k-tiles over D

SW = 16.0        # host scale on W_rep/b_rep fp8 (PSUM holds SW*embed)
SA = 8.0         # host scale on att fp8
GROUPS = [int(x) for x in os.environ.get("CSN_GROUPS", "3,3,3").split(",")]
assert sum(GROUPS) == NL
WARM_CC = os.environ.get("CSN_WARM_CC", "1") == "1"


def _split_multiwait_drains(nc):
    """This walrus build only accepts one sem wait per instruction; hoist
    extras onto NoOp carriers inserted just before the instruction (engines
    execute their stream in order, so wait-then-op is equivalent)."""
    fixno = 0
    for fnc in nc.m.functions:
        for bb in fnc.blocks:
            insts = bb.instructions
            i = 0
            while i < len(insts):
                inst = insts[i]
                si = inst.sync_info
                if si is not None and len(si.on_wait) > 1:
                    waits = list(si.on_wait)
                    si.on_wait = waits[-1:]
                    for w in waits[:-1]:
                        fixno += 1
                        carrier = mybir.InstNoOp(
                            name=f"I-waitfix-{fixno}",
                            engine=inst.engine,
                            ins=[],
                            outs=[],
                            sync_info=mybir.SyncInfo(on_wait=[w], on_update=[]),
                        )
                        insts.insert(i, carrier)
                        i += 1
                i += 1
    return fixno


def _n_of_r():
    """Exchange-row -> condition map: row r = R_OFF[g] + src*gs + i holds
    condition 9*src + N_OFF[g] + i."""
    GS = list(GROUPS)
    N_OFF = [sum(GS[:g]) for g in range(len(GS))]
    R_OFF = [NCORES * o for o in N_OFF]
    n_of_r = np.empty(NPAD, np.int64)
    for g in range(len(GS)):
        for src in range(NCORES):
            for i in range(GS[g]):
                n_of_r[R_OFF[g] + src * GS[g] + i] = NL * src + N_OFF[g] + i
    return n_of_r, GS, N_OFF, R_OFF


def _build():
    nc = bass.Bass(
        "TRN2", target_bir_lowering=False, debug=False, num_devices=NCORES
    )
    imgt = nc.dram_tensor("imgt", [128, KF * 128], BF16, kind="ExternalInput").ap()
    w_emb = nc.dram_tensor("w_emb", [KF, 128, D], BF16, kind="ExternalInput").ap()
    w_rep_l = nc.dram_tensor(
        "w_rep_l", [NL, KD // 2, 128, 2 * D], FP8, kind="ExternalInput"
    ).ap()
    attT = nc.dram_tensor("attT72", [NPAD, P], FP8, kind="ExternalInput").ap()
    out_shard = nc.dram_tensor(
        "out_shard", [BL, P, D], F32, kind="ExternalOutput"
    ).ap()
    x_out = nc.dram_tensor("x_out", [B, D], F32, kind="ExternalOutput").ap()

    GS = list(GROUPS)
    N_OFF = [sum(GS[:g]) for g in range(len(GS))]
    R_OFF = [NCORES * o for o in N_OFF]
    sends = [
        nc.dram_tensor(f"a2a_send{g}", [NCORES, gs, BL, D], FP8)
        for g, gs in enumerate(GS)
    ]
    recvs = [
        nc.dram_tensor(f"a2a_recv{g}", [NCORES, gs, BL, D], FP8)
        for g, gs in enumerate(GS)
    ]
    if WARM_CC:
        # deliberately uninitialized (content is never read): the trigger has
        # zero producer deps, so the CC engine's one-time mesh init (~50us)
        # starts at t~0, hidden under the GEMM instead of serializing in
        # front of the real exchanges.
        warm_s = nc.dram_tensor("warm_s", [NCORES, 16], F32)
        warm_r = nc.dram_tensor("warm_r", [NCORES, 16], F32)

    with tile.TileContext(nc) as tc, tc.tile_pool(name="const", bufs=1) as cpool:
        if WARM_CC:
            nc.gpsimd.collective_compute(
                "AllToAll",
                mybir.AluOpType.bypass,
                replica_groups=[list(range(NCORES))],
                ins=[warm_s[:].opt()],
                outs=[warm_r[:].opt()],
            )

        RINGS = [nc.sync, nc.scalar, nc.gpsimd]
        imgT_sb = cpool.tile([128, KF * 128], BF16, name="imgT_sb")
        nc.gpsimd.dma_start(imgT_sb[:], imgt[:])
        wemb_sb = cpool.tile([128, KF * D], BF16, name="wemb_sb")
        for k in range(KF):
            RINGS[k % 3].dma_start(
                wemb_sb[:, k * D : (k + 1) * D], w_emb[k, :, :]
            )
        attT_sb = cpool.tile([NPAD, P], FP8, name="attT_sb")
        nc.gpsimd.dma_start(attT_sb[:], attT[:])

        # ---- x = image @ W_emb (+b_emb via 17th k-tile) ------------------
        x_sb = cpool.tile([128, D], F32, name="x_sb")
        xT_sb = cpool.tile([128, D], FP8, name="xT_sb")  # 8 blocks [128d,128b]
        id_sb = cpool.tile([128, 128], F32, name="id_sb")
        from concourse.masks import make_identity

        make_identity(nc, id_sb[:])
        with (
            tc.tile_pool(name="xpsum", bufs=2, space="PSUM") as xpsum,
            tc.tile_pool(name="tpsum", bufs=2, space="PSUM") as tpsum,
        ):
            x_ps = [xpsum.tile([128, 512], F32, name=f"x_ps{h}") for h in range(2)]
            for k in range(KF):
                for h in range(2):
                    nc.tensor.matmul(
                        x_ps[h][:],
                        imgT_sb[:, k * 128 : (k + 1) * 128],
                        wemb_sb[:, k * D + h * 512 : k * D + (h + 1) * 512],
                        start=(k == 0),
                        stop=(k == KF - 1),
                    )
            for h in range(2):
                nc.vector.tensor_copy(
                    x_sb[:, h * 512 : (h + 1) * 512], x_ps[h][:]
                )
            nc.gpsimd.dma_start(x_out[:], x_sb[:])
            # preload the ACT engine's Copy table so the first reduce-phase
            # activation copy doesn't pay the ~1.5us table load
            actwarm = cpool.tile([1, 1], F32, name="actwarm")
            nc.scalar.activation(
                actwarm[:],
                id_sb[0:1, 0:1],
                mybir.ActivationFunctionType.Copy,
                scale=1.0 / (SW * SA),
            )
            for m in range(KD):
                tp = tpsum.tile([128, 128], F32, name="tp", tag="tp")
                nc.tensor.transpose(
                    tp[:], x_sb[:, m * 128 : (m + 1) * 128], id_sb[:]
                )
                nc.vector.tensor_copy(xT_sb[:, m * 128 : (m + 1) * 128], tp[:])

        # ---- grouped GEMM (fp8 DoubleRow) + pipelined exchange ----------
        r_sb = cpool.tile([NPAD, BL * D], FP8, name="r_sb")

        def exchange_group(g):
            gs = GS[g]
            rows = slice(R_OFF[g], R_OFF[g] + NCORES * gs)
            nc.gpsimd.collective_compute(
                "AllToAll",
                mybir.AluOpType.bypass,
                replica_groups=[list(range(NCORES))],
                ins=[sends[g][:].opt()],
                outs=[recvs[g][:].opt()],
            )
            nc.sync.dma_start(
                r_sb[rows, :],
                recvs[g][:].rearrange("c i b d -> (c i) (b d)"),
            )

        e_all = cpool.tile([128, NL * D], FP8, name="e_all")
        with (
            tc.tile_pool(name="wpool", bufs=8) as wpool,
            tc.tile_pool(name="gpool", bufs=3) as gpool,
            tc.tile_pool(name="cpsum", bufs=4, space="PSUM") as cpsum,
        ):
            # kp 0-2 stream on sync/scalar; kp 3 on gpsimd with a 2-condition
            # lookahead so it is never queued behind a send DMA that waits on
            # GEMM results (ring order: ... send(n), wt3(n+2), send(n+1) ...).
            wt3 = {}

            def load_wt3(n):
                wt3[n] = gpool.tile([128, 2 * D], FP8, name="wt3", tag="wt3")
                nc.gpsimd.dma_start(wt3[n][:], w_rep_l[n, 3, :, :])

            load_wt3(0)
            load_wt3(1)
            for n in range(NL):
                e_ps = [
                    cpsum.tile([128, 512], F32, name="e_ps", tag=f"e_ps{h}")
                    for h in range(2)
                ]
                for kp in range(KD // 2):
                    if kp == 3:
                        wt = wt3.pop(n)
                    else:
                        wt = wpool.tile([128, 2 * D], FP8, name="wt", tag="wt")
                        eng = [nc.sync, nc.scalar, nc.sync if n % 2 else nc.scalar][kp]
                        eng.dma_start(wt[:], w_rep_l[n, kp, :, :])
                    lhs = xT_sb[:, 2 * kp * 128 : (2 * kp + 2) * 128].rearrange(
                        "p (i b) -> p i b", i=2
                    )
                    wv = wt[:].rearrange("p (i d) -> p i d", i=2)
                    for h in range(2):
                        nc.tensor.matmul(
                            e_ps[h][:],
                            lhs,
                            wv[:, :, h * 512 : (h + 1) * 512],
                            start=(kp == 0),
                            stop=(kp == KD // 2 - 1),
                            perf_mode=mybir.MatmulPerfMode.DoubleRow,
                        )
                e_sb = e_all[:, n * D : (n + 1) * D]
                for h in range(2):
                    nc.vector.tensor_copy(
                        e_sb[:, h * 512 : (h + 1) * 512], e_ps[h][:]
                    )
                g = max(i for i in range(len(GS)) if N_OFF[i] <= n)
                nc.gpsimd.dma_start(sends[g][:, n - N_OFF[g], :, :], e_sb)
                if n + 2 < NL:
                    load_wt3(n + 2)
                if n - N_OFF[g] == GS[g] - 1:
                    exchange_group(g)

        # ---- attention reduce: out = (attT/8).T @ (r/16) ----------------
        with (
            tc.tile_pool(name="rpsum", bufs=6, space="PSUM") as rpsum,
            tc.tile_pool(name="spool", bufs=6) as spool,
        ):
            for j in range(BL * D // 512):
                o_ps = rpsum.tile([P, 512], F32, name="o_ps", tag="o_ps")
                nc.tensor.matmul(
                    o_ps[:],
                    attT_sb[:],
                    r_sb[:, j * 512 : (j + 1) * 512],
                    start=True,
                    stop=True,
                )
                stg = spool.tile([P, 512], F32, name="stg", tag="stg")
                if j % 2 == 0:
                    nc.vector.tensor_scalar_mul(stg[:], o_ps[:], 1.0 / (SW * SA))
                else:
                    nc.scalar.activation(
                        stg[:],
                        o_ps[:],
                        mybir.ActivationFunctionType.Copy,
                        scale=1.0 / (SW * SA),
                    )
                RINGS[j % 3].dma_start(
                    out_shard[j // 2, :, (j % 2) * 512 : (j % 2 + 1) * 512],
                    stg[:],
                )

    if os.environ.get("CSN_NO_WAITFIX", "0") != "1":
        _split_multiwait_drains(nc)
    return nc


_NC_CACHE = {}
_LAST_IN_MAPS = None


def _get_nc():
    key = (tuple(GROUPS), WARM_CC)
    if key not in _NC_CACHE:
        _NC_CACHE[key] = _build()
    return _NC_CACHE[key]


def kernel(image, W_emb, b_emb, W_rep, b_rep, mask_table, W1, b1, W2, b2, cat_enc):
    import ml_dtypes

    f8 = ml_dtypes.float8_e4m3
    bf = ml_dtypes.bfloat16

    image = np.asarray(image, np.float32)
    W_emb = np.asarray(W_emb, np.float32)
    b_emb = np.asarray(b_emb, np.float32).reshape(D)
    W_rep = np.asarray(W_rep, np.float32)
    b_rep = np.asarray(b_rep, np.float32)
    mask_table = np.asarray(mask_table, np.float32)
    W1 = np.asarray(W1, np.float32)
    b1 = np.asarray(b1, np.float32).reshape(N)
    W2 = np.asarray(W2, np.float32)
    b2 = np.asarray(b2, np.float32).reshape(N)
    cat_enc = np.asarray(cat_enc, np.float32)

    # host att (input-only): softmax(relu(cat_enc@W1+b1)@W2+b2)
    h = np.maximum(cat_enc @ W1 + b1, 0.0)
    lg = h @ W2 + b2
    e = np.exp(lg - lg.max(-1, keepdims=True))
    att = (e / e.sum(-1, keepdims=True)).astype(np.float32)      # [P, N]

    # fold mask into W/b; pad conditions to 72
    Wm = np.zeros((NPAD, D, D), np.float32)
    Wm[:N] = W_rep * mask_table[:, None, :]
    bm = b_rep * mask_table                                       # [N, D]

    # imgT with bias k-tile: imgT_sb[p, k*128+b] = image[b, k*128+p]
    imgt = np.zeros((128, KF * 128), np.float32)
    imgt[:, : FI] = (
        image.reshape(128, FI // 128, 128).transpose(2, 1, 0).reshape(128, FI)
    )
    imgt[0, FI:] = 1.0  # ones row for the b_emb k-tile
    # W_emb k-tiles + bias tile
    wemb = np.zeros((KF, 128, D), np.float32)
    wemb[: KF - 1] = W_emb.reshape(KF - 1, 128, D)
    wemb[KF - 1, 0] = b_emb

    # W_rep per (n, kp) slab [2*128, D] -> [128, (i d)] rows interleaved
    # w_host[n, kp, p, i*D+d] = SW * Wm[n, 2*kp*128 + i*128 + p, d]
    w_host = (SW * Wm).reshape(NPAD, KD // 2, 2, 128, D).transpose(0, 1, 3, 2, 4)
    w_host = np.ascontiguousarray(w_host).reshape(NPAD, KD // 2, 128, 2 * D)
    w_host = w_host.astype(f8)

    n_of_r, GS, N_OFF, R_OFF = _n_of_r()
    attT72 = np.zeros((NPAD, P), np.float32)
    for r in range(NPAD):
        if n_of_r[r] < N:
            attT72[r] = SA * att[:, n_of_r[r]]
    attT72 = attT72.astype(f8)

    nc = _get_nc()
    in_maps = []
    for c in range(NCORES):
        m = {
            "imgt": imgt.astype(bf),
            "w_emb": wemb.astype(bf),
            "w_rep_l": np.ascontiguousarray(w_host[c * NL : (c + 1) * NL]),
            "attT72": attT72,
        }
        in_maps.append(m)

    global _LAST_IN_MAPS
    _LAST_IN_MAPS = in_maps
    res = run_bass_kernel_spmd(nc, in_maps, list(range(NCORES)))

    out = np.empty((B, P + N, D), np.float32)
    out[:, :P] = np.concatenate(
        [res.results[c]["out_shard"] for c in range(NCORES)], axis=0
    )
    out[:, :P] += (att @ bm)[None, :, :]          # b_rep contribution (zero here)
    out[:, P:] = res.results[0]["x_out"][:, None, :]
    return out


# revision 30
# speedup vs baseline: 1.4660x; 1.4660x over previous
"""Trainium2 Bass kernel for ConditionalSimNet2 (moe_routing).

Computation (B=128, FEAT_IN=2048, D=1024, N=P=66 conditions):
    x          = image @ W_emb + b_emb                    [B, D]
    masked_rep = einsum('bd,nde->bne', x, W_rep) + b_rep  [B, N, D]
    embed      = mask_table * masked_rep                  [B, N, D]
    att        = softmax(relu(cat_enc@W1+b1)@W2 + b2)     [P, N]
    cond_feat  = einsum('pn,bnd->bpd', att, embed)        [B, P, D]
    out        = concat([cond_feat, broadcast(x)], 1)     [B, P+N, D]

Device work is only the big GEMMs; everything input-only is host math:
  - mask_table is folded into W_rep columns / b_rep on the host.
  - att (66x66, input-only) is computed on the host; the device receives
    attT72 = 8*att permuted into exchange-row order.
  - b_rep's contribution att@ (mask*b_rep) is a batch-independent [P, D]
    matrix added on the host (it is exactly zero for this model).
  - b_emb rides as a 17th k-tile of the x GEMM (host-padded W_emb/imgT).

Sharding: expert-parallel over 66->72 conditions, 9 per core.  Every
core computes x redundantly (bf16), runs its 9 grouped GEMMs in fp8
DoubleRow (W pre-scaled x16 into e4m3 on host), exchanges embed slices
in fp8 via 3 pipelined AllToAlls (a tiny warm-up AllToAll at t=0
absorbs core launch skew), then reduces its 16-row batch shard with a
single fp8 matmul per 512-col slice (PSUM = 128*cond_feat, descaled in
the PSUM->SBUF copy).  Host concatenates the batch shards and
broadcasts x into the feature_x half.
"""

import os
import sys

import numpy as np

try:
    import concourse.bass as bass
except ImportError:  # pragma: no cover - fallback when PYTHONPATH is not set
    sys.path.insert(0, "/opt/trn_rl_repo")
    import concourse.bass as bass

import concourse.mybir as mybir
import concourse.tile as tile
from concourse.bass_utils import run_bass_kernel_spmd

F32 = mybir.dt.float32
BF16 = mybir.dt.bfloat16
FP8 = mybir.dt.float8e4

B = 128          # batch
FI = 2048        # backbone feature dim
D = 1024         # embed dim
N = 66           # conditions (== pair categories P)
P = 66
NCORES = 8
NL = 9           # conditions per core (66 -> 72 padded)
NPAD = NCORES * NL
BL = B // NCORES  # batch rows per core
KF = FI // 128 + 1  # 16 k-tiles over FEAT_IN + 1 bias tile
KD = D // 128       # 8 k-tiles over D

SW = 16.0        # host scale on W_rep/b_rep fp8 (PSUM holds SW*embed)
SA = 8.0         # host scale on att fp8
GROUPS = [int(x) for x in os.environ.get("CSN_GROUPS", "3,3,3").split(",")]
assert sum(GROUPS) == NL
WARM_CC = os.environ.get("CSN_WARM_CC", "1") == "1"


def _split_multiwait_drains(nc):
    """This walrus build only accepts one sem wait per instruction; hoist
    extras onto NoOp carriers inserted just before the instruction (engines
    execute their stream in order, so wait-then-op is equivalent)."""
    fixno = 0
    for fnc in nc.m.functions:
        for bb in fnc.blocks:
            insts = bb.instructions
            i = 0
            while i < len(insts):
                inst = insts[i]
                si = inst.sync_info
                if si is not None and len(si.on_wait) > 1:
                    waits = list(si.on_wait)
                    si.on_wait = waits[-1:]
                    for w in waits[:-1]:
                        fixno += 1
                        carrier = mybir.InstNoOp(
                            name=f"I-waitfix-{fixno}",
                            engine=inst.engine,
                            ins=[],
                            outs=[],
                            sync_info=mybir.SyncInfo(on_wait=[w], on_update=[]),
                        )
                        insts.insert(i, carrier)
                        i += 1
                i += 1
    return fixno


def _n_of_r():
    """Exchange-row -> condition map: row r = R_OFF[g] + src*gs + i holds
    condition 9*src + N_OFF[g] + i."""
    GS = list(GROUPS)
    N_OFF = [sum(GS[:g]) for g in range(len(GS))]
    R_OFF = [NCORES * o for o in N_OFF]
    n_of_r = np.empty(NPAD, np.int64)
    for g in range(len(GS)):
        for src in range(NCORES):
            for i in range(GS[g]):
                n_of_r[R_OFF[g] + src * GS[g] + i] = NL * src + N_OFF[g] + i
    return n_of_r, GS, N_OFF, R_OFF


def _build():
    nc = bass.Bass(
        "TRN2", target_bir_lowering=False, debug=False, num_devices=NCORES
    )
    imgt = nc.dram_tensor("imgt", [128, KF * 128], BF16, kind="ExternalInput").ap()
    w_emb = nc.dram_tensor("w_emb", [KF, 128, D], BF16, kind="ExternalInput").ap()
    w_rep_l = nc.dram_tensor(
        "w_rep_l", [NL, KD // 2, 128, 2 * D], FP8, kind="ExternalInput"
    ).ap()
    attT = nc.dram_tensor("attT72", [NPAD, P], FP8, kind="ExternalInput").ap()
    out_shard = nc.dram_tensor(
        "out_shard", [BL, P, D], F32, kind="ExternalOutput"
    ).ap()
    x_out = nc.dram_tensor("x_out", [B, D], F32, kind="ExternalOutput").ap()

    GS = list(GROUPS)
    N_OFF = [sum(GS[:g]) for g in range(len(GS))]
    R_OFF = [NCORES * o for o in N_OFF]
    sends = [
        nc.dram_tensor(f"a2a_send{g}", [NCORES, gs, BL, D], FP8)
        for g, gs in enumerate(GS)
    ]
    recvs = [
        nc.dram_tensor(f"a2a_recv{g}", [NCORES, gs, BL, D], FP8)
        for g, gs in enumerate(GS)
    ]
    if WARM_CC:
        # dummy exchange triggered at t~0: the CC engine's one-time mesh
        # init (~40us) runs hidden under the GEMM instead of serializing in
        # front of the real exchanges.  warm_s needs a real writer — with no
        # writer at all the CC engine stalls ~80us on the unmaterialized
        # buffer.
        warm_s = nc.dram_tensor("warm_s", [NCORES, 16], F32)
        warm_r = nc.dram_tensor("warm_r", [NCORES, 16], F32)

    with tile.TileContext(nc) as tc, tc.tile_pool(name="const", bufs=1) as cpool:
        if WARM_CC:
            warm_sb = cpool.tile([NCORES, 16], F32, name="warm_sb")
            nc.gpsimd.memset(warm_sb[:], 0.0)
            nc.gpsimd.dma_start(warm_s[:], warm_sb[:])
            nc.gpsimd.collective_compute(
                "AllToAll",
                mybir.AluOpType.bypass,
                replica_groups=[list(range(NCORES))],
                ins=[warm_s[:].opt()],
                outs=[warm_r[:].opt()],
            )

        RINGS = [nc.sync, nc.scalar, nc.gpsimd]
        imgT_sb = cpool.tile([128, KF * 128], BF16, name="imgT_sb")
        nc.gpsimd.dma_start(imgT_sb[:], imgt[:])
        wemb_sb = cpool.tile([128, KF * D], BF16, name="wemb_sb")
        for k in range(KF):
            RINGS[k % 3].dma_start(
                wemb_sb[:, k * D : (k + 1) * D], w_emb[k, :, :]
            )
        attT_sb = cpool.tile([NPAD, P], FP8, name="attT_sb")
        nc.gpsimd.dma_start(attT_sb[:], attT[:])

        # ---- x = image @ W_emb (+b_emb via 17th k-tile) ------------------
        x_sb = cpool.tile([128, D], F32, name="x_sb")
        xT_sb = cpool.tile([128, D], FP8, name="xT_sb")  # 8 blocks [128d,128b]
        id_sb = cpool.tile([128, 128], F32, name="id_sb")
        from concourse.masks import make_identity

        make_identity(nc, id_sb[:])
        with (
            tc.tile_pool(name="xpsum", bufs=2, space="PSUM") as xpsum,
            tc.tile_pool(name="tpsum", bufs=2, space="PSUM") as tpsum,
        ):
            x_ps = [xpsum.tile([128, 512], F32, name=f"x_ps{h}") for h in range(2)]
            for k in range(KF):
                for h in range(2):
                    nc.tensor.matmul(
                        x_ps[h][:],
                        imgT_sb[:, k * 128 : (k + 1) * 128],
                        wemb_sb[:, k * D + h * 512 : k * D + (h + 1) * 512],
                        start=(k == 0),
                        stop=(k == KF - 1),
                    )
            for h in range(2):
                nc.vector.tensor_copy(
                    x_sb[:, h * 512 : (h + 1) * 512], x_ps[h][:]
                )
            nc.gpsimd.dma_start(x_out[:], x_sb[:])
            # preload the ACT engine's Copy table so the first reduce-phase
            # activation copy doesn't pay the ~1.5us table load
            actwarm = cpool.tile([1, 1], F32, name="actwarm")
            nc.scalar.activation(
                actwarm[:],
                id_sb[0:1, 0:1],
                mybir.ActivationFunctionType.Copy,
                scale=1.0 / (SW * SA),
            )
            for m in range(KD):
                tp = tpsum.tile([128, 128], F32, name="tp", tag="tp")
                nc.tensor.transpose(
                    tp[:], x_sb[:, m * 128 : (m + 1) * 128], id_sb[:]
                )
                nc.vector.tensor_copy(xT_sb[:, m * 128 : (m + 1) * 128], tp[:])

        # ---- grouped GEMM (fp8 DoubleRow) + pipelined exchange ----------
        r_sb = cpool.tile([NPAD, BL * D], FP8, name="r_sb")

        def exchange_group(g):
            gs = GS[g]
            rows = slice(R_OFF[g], R_OFF[g] + NCORES * gs)
            nc.gpsimd.collective_compute(
                "AllToAll",
                mybir.AluOpType.bypass,
                replica_groups=[list(range(NCORES))],
                ins=[sends[g][:].opt()],
                outs=[recvs[g][:].opt()],
            )
            nc.sync.dma_start(
                r_sb[rows, :],
                recvs[g][:].rearrange("c i b d -> (c i) (b d)"),
            )

        e_all = cpool.tile([128, NL * D], FP8, name="e_all")
        with (
            tc.tile_pool(name="wpool", bufs=8) as wpool,
            tc.tile_pool(name="gpool", bufs=3) as gpool,
            tc.tile_pool(name="cpsum", bufs=4, space="PSUM") as cpsum,
        ):
            # kp 0-2 stream on sync/scalar; kp 3 on gpsimd with a 2-condition
            # lookahead so it is never queued behind a send DMA that waits on
            # GEMM results (ring order: ... send(n), wt3(n+2), send(n+1) ...).
            wt3 = {}

            def load_wt3(n):
                wt3[n] = gpool.tile([128, 2 * D], FP8, name="wt3", tag="wt3")
                nc.gpsimd.dma_start(wt3[n][:], w_rep_l[n, 3, :, :])

            load_wt3(0)
            load_wt3(1)
            for n in range(NL):
                e_ps = [
                    cpsum.tile([128, 512], F32, name="e_ps", tag=f"e_ps{h}")
                    for h in range(2)
                ]
                for kp in range(KD // 2):
                    if kp == 3:
                        wt = wt3.pop(n)
                    else:
                        wt = wpool.tile([128, 2 * D], FP8, name="wt", tag="wt")
                        eng = [nc.sync, nc.scalar, nc.sync if n % 2 else nc.scalar][kp]
                        eng.dma_start(wt[:], w_rep_l[n, kp, :, :])
                    lhs = xT_sb[:, 2 * kp * 128 : (2 * kp + 2) * 128].rearrange(
                        "p (i b) -> p i b", i=2
                    )
                    wv = wt[:].rearrange("p (i d) -> p i d", i=2)
                    for h in range(2):
                        nc.tensor.matmul(
                            e_ps[h][:],
                            lhs,
                            wv[:, :, h * 512 : (h + 1) * 512],
                            start=(kp == 0),
                            stop=(kp == KD // 2 - 1),
                            perf_mode=mybir.MatmulPerfMode.DoubleRow,
                        )
                e_sb = e_all[:, n * D : (n + 1) * D]
                for h in range(2):
                    nc.vector.tensor_copy(
                        e_sb[:, h * 512 : (h + 1) * 512], e_ps[h][:]
                    )
                g = max(i for i in range(len(GS)) if N_OFF[i] <= n)
                nc.gpsimd.dma_start(sends[g][:, n - N_OFF[g], :, :], e_sb)
                if n + 2 < NL:
                    load_wt3(n + 2)
                if n - N_OFF[g] == GS[g] - 1:
                    exchange_group(g)

        # ---- attention reduce: out = (attT/8).T @ (r/16) ----------------
        with (
            tc.tile_pool(name="rpsum", bufs=6, space="PSUM") as rpsum,
            tc.tile_pool(name="spool", bufs=6) as spool,
        ):
            for j in range(BL * D // 512):
                o_ps = rpsum.tile([P, 512], F32, name="o_ps", tag="o_ps")
                nc.tensor.matmul(
                    o_ps[:],
                    attT_sb[:],
                    r_sb[:, j * 512 : (j + 1) * 512],
                    start=True,
                    stop=True,
                )
                stg = spool.tile([P, 512], F32, name="stg", tag="stg")
                if j % 2 == 0:
                    nc.vector.tensor_scalar_mul(stg[:], o_ps[:], 1.0 / (SW * SA))
                else:
                    nc.scalar.activation(
                        stg[:],
                        o_ps[:],
                        mybir.ActivationFunctionType.Copy,
                        scale=1.0 / (SW * SA),
                    )
                RINGS[j % 3].dma_start(
                    out_shard[j // 2, :, (j % 2) * 512 : (j % 2 + 1) * 512],
                    stg[:],
                )

    if os.environ.get("CSN_NO_WAITFIX", "0") != "1":
        _split_multiwait_drains(nc)
    return nc


_NC_CACHE = {}
_LAST_IN_MAPS = None


def _get_nc():
    key = (tuple(GROUPS), WARM_CC)
    if key not in _NC_CACHE:
        _NC_CACHE[key] = _build()
    return _NC_CACHE[key]


def kernel(image, W_emb, b_emb, W_rep, b_rep, mask_table, W1, b1, W2, b2, cat_enc):
    import ml_dtypes

    f8 = ml_dtypes.float8_e4m3
    bf = ml_dtypes.bfloat16

    image = np.asarray(image, np.float32)
    W_emb = np.asarray(W_emb, np.float32)
    b_emb = np.asarray(b_emb, np.float32).reshape(D)
    W_rep = np.asarray(W_rep, np.float32)
    b_rep = np.asarray(b_rep, np.float32)
    mask_table = np.asarray(mask_table, np.float32)
    W1 = np.asarray(W1, np.float32)
    b1 = np.asarray(b1, np.float32).reshape(N)
    W2 = np.asarray(W2, np.float32)
    b2 = np.asarray(b2, np.float32).reshape(N)
    cat_enc = np.asarray(cat_enc, np.float32)

    # host att (input-only): softmax(relu(cat_enc@W1+b1)@W2+b2)
    h = np.maximum(cat_enc @ W1 + b1, 0.0)
    lg = h @ W2 + b2
    e = np.exp(lg - lg.max(-1, keepdims=True))
    att = (e / e.sum(-1, keepdims=True)).astype(np.float32)      # [P, N]

    # fold mask into W/b; pad conditions to 72
    Wm = np.zeros((NPAD, D, D), np.float32)
    Wm[:N] = W_rep * mask_table[:, None, :]
    bm = b_rep * mask_table                                       # [N, D]

    # imgT with bias k-tile: imgT_sb[p, k*128+b] = image[b, k*128+p]
    imgt = np.zeros((128, KF * 128), np.float32)
    imgt[:, : FI] = (
        image.reshape(128, FI // 128, 128).transpose(2, 1, 0).reshape(128, FI)
    )
    imgt[0, FI:] = 1.0  # ones row for the b_emb k-tile
    # W_emb k-tiles + bias tile
    wemb = np.zeros((KF, 128, D), np.float32)
    wemb[: KF - 1] = W_emb.reshape(KF - 1, 128, D)
    wemb[KF - 1, 0] = b_emb

    # W_rep per (n, kp) slab [2*128, D] -> [128, (i d)] rows interleaved
    # w_host[n, kp, p, i*D+d] = SW * Wm[n, 2*kp*128 + i*128 + p, d]
    w_host = (SW * Wm).reshape(NPAD, KD // 2, 2, 128, D).transpose(0, 1, 3, 2, 4)
    w_host = np.ascontiguousarray(w_host).reshape(NPAD, KD // 2, 128, 2 * D)
    w_host = w_host.astype(f8)

    n_of_r, GS, N_OFF, R_OFF = _n_of_r()
    attT72 = np.zeros((NPAD, P), np.float32)
    for r in range(NPAD):
        if n_of_r[r] < N:
            attT72[r] = SA * att[:, n_of_r[r]]
    attT72 = attT72.astype(f8)

    nc = _get_nc()
    in_maps = []
    for c in range(NCORES):
        m = {
            "imgt": imgt.astype(bf),
            "w_emb": wemb.astype(bf),
            "w_rep_l": np.ascontiguousarray(w_host[c * NL : (c + 1) * NL]),
            "attT72": attT72,
        }
        in_maps.append(m)

    global _LAST_IN_MAPS
    _LAST_IN_MAPS = in_maps
    res = run_bass_kernel_spmd(nc, in_maps, list(range(NCORES)))

    out = np.empty((B, P + N, D), np.float32)
    out[:, :P] = np.concatenate(
        [res.results[c]["out_shard"] for c in range(NCORES)], axis=0
    )
    out[:, :P] += (att @ bm)[None, :, :]          # b_rep contribution (zero here)
    out[:, P:] = res.results[0]["x_out"][:, None, :]
    return out
